# revision 11
# baseline (speedup 1.0000x reference)
"""Trainium2 Bass kernel for nn_CrossAttention (B=8, N=4096, S=512, D=512, H=8).

Sharding: data-parallel over batch - each of the 8 NeuronCores computes the
full cross-attention for one batch element. No collectives needed.

v2 design (vs the v1 baseline at ~320us), driven by the NTFF profile of v1:
  - The PE clock ramps (0.65 -> 1.2 -> 2.4 GHz after 3us of CONTINUOUS
    execution); any stall resets it. v1 had 72 PE busy-intervals so ~40% of
    matmuls ran at 1.2GHz (634ns vs 385ns per 512-col matmul). v2 arranges
    the PE stream as uniform 10-matmul head slots so the PE never waits:
      slot(h) = [scores(h) x3 | attnV(h-1) x3 | filler x4]
    with fillers = qproj(t+1) K-chains (slots 0-3) and outproj(t-1)
    K-chains (slots 4-7). The softmax-denominator chain of tile t has
    ~10us of slack because outproj(t) only starts mid-way into tile t+1.
  - Masking costs nothing on-chip: masked/padded context rows are zeroed
    HOST-side, and the denominator "ones" column in the attnV stationary is
    the 0/1 unmasked indicator. exp then needs no bias, so two score chunks
    share one ACT instruction over a 2-bank PSUM tile (ACT is the #2 engine;
    24 exps/tile -> 16 ACT ops/tile).
  - Softmax denominators for all 8 heads are DMA-gathered into a [128, 32]
    tile (v1's [8, 512] DVE reciprocal cost 3.3us; reshaped it is ~0.4us),
    inverted once, and broadcast back over partitions via a bf16 DRAM
    bounce with two stride-0 mid-dim DMAs.
  - ACT does ONLY exp (v1 had 38us of DMA descriptor-gen on the scalar
    queue); DMA triggers live on the sync + gpsimd queues; PSUM evacuations
    on DVE; normalization muls split DVE/gpsimd.

All linear-layer biases are zero in this problem; the kernel asserts that
and skips them entirely.
"""

import os

import numpy as np

try:
    import concourse.bass as bass
except ImportError:
    import sys

    sys.path.insert(0, "/opt/trn_rl_repo")
    import concourse.bass as bass

from contextlib import ExitStack

import concourse.mybir as mybir
import concourse.tile as tile
from concourse.bass import ts

B, N, S, D, H = 8, 4096, 512, 512, 8
HD = D // H  # 64
SCALE = HD**-0.5
P = 128
IC = D // P  # 4 chunks of feature dims
NT = 512  # queries per outer tile
NTILES = N // NT  # 8
NSUB = NT // P  # 4

f32 = mybir.dt.float32

MMDT_NAME = os.environ.get("KMMDT", "bfloat16")
# pair two score chunks per ACT exp (needs the 2-bank PSUM tile)
PAIR_EXP = os.environ.get("KPAIR", "1") == "1"


def _np_mm(mmdt):
    return np.dtype(mybir.dt.np(mmdt))


def _split_multi_waits(nc: bass.Bass) -> None:
    """This walrus toolchain accepts at most ONE sync-wait per instruction
    ("Too many sync wait commands" in setupSyncWait, seen for MM/LW, NoOp,
    and DMA structs alike). Hoist all but the last wait of any instruction
    onto a chain of same-engine InstNoOps spliced immediately before it -
    same program position, so synchronization semantics are unchanged."""
    eng_map = {
        mybir.EngineType.PE: lambda: nc.tensor,
        mybir.EngineType.Activation: lambda: nc.scalar,
        mybir.EngineType.DVE: lambda: nc.vector,
        mybir.EngineType.Pool: lambda: nc.gpsimd,
        mybir.EngineType.SP: lambda: nc.sync,
    }
    for fn in nc.m.functions:
        blocks = fn.blocks
        for bb in blocks:
            insts = list(bb.instructions)
            out = []
            changed = False
            for inst in insts:
                si = inst.sync_info
                if (
                    si is not None
                    and len(si.on_wait) > 1
                    and inst.engine in eng_map
                ):
                    waits = list(si.on_wait)
                    for w in waits[:-1]:  # one nop per excess wait
                        nop = eng_map[inst.engine]().nop(nofuse=True).ins
                        # the nop was appended to whatever block is current;
                        # strip it from there before splicing it in place
                        for bb2 in blocks:
                            lst = list(bb2.instructions)
                            if any(x.name == nop.name for x in lst):
                                bb2.instructions = [
                                    x for x in lst if x.name != nop.name
                                ]
                                if bb2 is bb:
                                    insts = [
                                        x for x in insts if x.name != nop.name
                                    ]
                        nop.sync_info = mybir.SyncInfo(
                            on_wait=[w], on_update=[]
                        )
                        out.append(nop)
                    inst.sync_info = mybir.SyncInfo(
                        on_wait=waits[-1:], on_update=list(si.on_update)
                    )
                    changed = True
                out.append(inst)
            if changed:
                bb.instructions = out


def _build_nc(mmdt_name: str, SCc: int, pair_exp: bool) -> bass.Bass:
    mmdt = getattr(mybir.dt, mmdt_name)
    Sc = SCc * P
    SCP = (SCc + 1) // 2  # attnV ctx pair-slots (last may be unpaired)
    NPAIR = SCc // 2  # full exp pairs
    assert 2 <= SCc <= 4

    nc = bass.Bass()

    xT = nc.dram_tensor("xT", [D, N], mmdt, kind="ExternalInput")
    ctxT = nc.dram_tensor("ctxT", [D, Sc], mmdt, kind="ExternalInput")
    wqT = nc.dram_tensor("wqT", [D, D], mmdt, kind="ExternalInput")
    wkT = nc.dram_tensor("wkT", [D, D], mmdt, kind="ExternalInput")
    wvT = nc.dram_tensor("wvT", [D, D], mmdt, kind="ExternalInput")
    wpT = nc.dram_tensor("wpT", [D, D], mmdt, kind="ExternalInput")
    uns = nc.dram_tensor("uns", [Sc, 1], mmdt, kind="ExternalInput")
    y = nc.dram_tensor("y", [N, D], mmdt, kind="ExternalOutput")

    # per-tile reciprocal-denominator bounce, flat (h, n) == (p, j) order
    rden_dram = nc.dram_tensor("rden_scratch", [NTILES, H * NT], mmdt)
    rden_w = rden_dram.rearrange("t (p j) -> t p j", p=P)  # [T, 128, 32]
    rden_r = rden_dram.rearrange("t (c par n) -> t par c n", par=2, n=NT)

    ch = lambda dram: dram.rearrange("(c p) o -> p c o", p=P)  # [P, IC, D]

    with tile.TileContext(nc) as tc, ExitStack() as ctx:
        const = ctx.enter_context(tc.tile_pool(name="const", bufs=1))
        work = ctx.enter_context(tc.tile_pool(name="work", bufs=2))
        epool = ctx.enter_context(tc.tile_pool(name="epool", bufs=3))
        psum = ctx.enter_context(tc.tile_pool(name="psum", bufs=1, space="PSUM"))

        # ---- persistent tiles -------------------------------------------
        wq_t = const.tile([P, IC, D], mmdt)
        wk_t = const.tile([P, IC, D], mmdt)
        wv_t = const.tile([P, IC, D], mmdt)
        wp_t = const.tile([P, IC, D], mmdt)
        ctx_t = const.tile([P, IC, Sc], mmdt)
        kT_t = const.tile([P, IC, Sc], mmdt)
        # token-major v (+ indicator col 64 for the softmax denominator),
        # per head, ctx chunks paired along the free dim
        vext_t = const.tile([P, H, SCP, 2, HD + 1], mmdt)

        # startup DMAs spread over all five queue engines so the transfers
        # overlap (concurrent transfers share the 16 DMA engines; a single
        # 512KB transfer can take >20us when the fabric is busy)
        nc.sync.dma_start(wq_t[:], ch(wqT))
        nc.gpsimd.dma_start(ctx_t[:], ctxT.rearrange("(c p) s -> p c s", p=P))
        nc.scalar.dma_start(wk_t[:], ch(wkT))
        for sc in range(SCc):  # indicator column (one tiny DMA per chunk)
            nc.gpsimd.dma_start(
                vext_t[:, :, sc // 2, sc % 2, HD : HD + 1],
                uns[sc * P : (sc + 1) * P, :][:, None, :].to_broadcast(
                    (P, H, 1)
                ),
            )
        nc.gpsimd.dma_start(wv_t[:], ch(wvT))

        def load_xT(tt):
            xt = work.tile([P, IC, NT], mmdt, tag="xT", bufs=3)
            nc.sync.dma_start(
                xt[:], xT[:, ts(tt, NT)].rearrange("(c p) n -> p c n", p=P)
            )
            return xt

        xT_cur = load_xT(0)
        xT_next = load_xT(1)
        nc.gpsimd.dma_start(wp_t[:], ch(wpT))

        def qproj_chunk(xT_t, qm_t, oc):
            ps = psum.tile([P, NT], f32, tag="ps_q", bufs=2)
            for i in range(IC):
                nc.tensor.matmul(
                    ps[:],
                    wq_t[:, i, ts(oc, P)],
                    xT_t[:, i, :],
                    start=(i == 0),
                    stop=(i == IC - 1),
                )
            nc.vector.tensor_copy(qm_t[:, oc, :], ps[:])

        # ---- startup: qproj(0), then kv projections ---------------------
        qm_cur = work.tile([P, IC, NT], mmdt, tag="qm")
        for oc in range(IC):
            qproj_chunk(xT_cur, qm_cur, oc)

        for kc in range(IC):  # dk chunks -> kT (feature-major keys)
            ps = psum.tile([P, NT], f32, tag="ps_q", bufs=2)
            for i in range(IC):
                nc.tensor.matmul(
                    ps[:, 0:Sc],
                    wk_t[:, i, ts(kc, P)],
                    ctx_t[:, i, :],
                    start=(i == 0),
                    stop=(i == IC - 1),
                )
            nc.vector.tensor_copy(kT_t[:, kc, :], ps[:, 0:Sc])

        for sc in range(SCc):  # s chunks -> v (token-major)
            # alternate attnV banks so chunk sc+1's matmuls don't wait for
            # chunk sc's 8 evacuation copies
            ps = psum.tile(
                [P, D], f32, tag="ps_oe" if sc % 2 == 0 else "ps_oo", bufs=1
            )
            for i in range(IC):
                nc.tensor.matmul(
                    ps[:],
                    ctx_t[:, i, ts(sc, P)],
                    wv_t[:, i, :],
                    start=(i == 0),
                    stop=(i == IC - 1),
                )
            for h in range(H):
                nc.vector.tensor_copy(
                    vext_t[:, h, sc // 2, sc % 2, 0:HD],
                    ps[:, h * HD : (h + 1) * HD],
                )

        # ---- per-tile emission helpers ----------------------------------
        def scores_head(qm_t, e8, h):
            pslc = slice((h % 2) * HD, (h % 2 + 1) * HD)
            c = h // 2
            if pair_exp:
                for pr in range(NPAIR):
                    ps2 = psum.tile([P, 2, NT], f32, tag="ps_s2", bufs=1)
                    for j in range(2):
                        sc = 2 * pr + j
                        nc.tensor.matmul(
                            ps2[:, j, :],
                            kT_t[pslc, c, ts(sc, P)],
                            qm_t[pslc, c, :],
                            start=True,
                            stop=True,
                        )
                    nc.scalar.activation(
                        e8[:, pr, :, :],
                        ps2[:],
                        mybir.ActivationFunctionType.Exp,
                        scale=SCALE,
                    )
                if SCc % 2:
                    sc = SCc - 1
                    ps1 = psum.tile([P, NT], f32, tag="ps_s1", bufs=1)
                    nc.tensor.matmul(
                        ps1[:],
                        kT_t[pslc, c, ts(sc, P)],
                        qm_t[pslc, c, :],
                        start=True,
                        stop=True,
                    )
                    nc.scalar.activation(
                        e8[:, sc // 2, sc % 2, :],
                        ps1[:],
                        mybir.ActivationFunctionType.Exp,
                        scale=SCALE,
                    )
            else:
                for sc in range(SCc):
                    ps1 = psum.tile([P, NT], f32, tag="ps_s1", bufs=3)
                    nc.tensor.matmul(
                        ps1[:],
                        kT_t[pslc, c, ts(sc, P)],
                        qm_t[pslc, c, :],
                        start=True,
                        stop=True,
                    )
                    nc.scalar.activation(
                        e8[:, sc // 2, sc % 2, :],
                        ps1[:],
                        mybir.ActivationFunctionType.Exp,
                        scale=SCALE,
                    )

        def attnv_head(e8, oe_t, stag_t, h):
            # rows 0:64 = unnormalized out, row 64 = denominator
            ps_o = psum.tile(
                [P, NT], f32, tag="ps_oe" if h % 2 == 0 else "ps_oo", bufs=1
            )
            nmm = 0
            for scp in range(SCP):
                for i in range(2):
                    if 2 * scp + i >= SCc:
                        break
                    nc.tensor.matmul(
                        ps_o[0 : HD + 1, :],
                        vext_t[:, h, scp, i, :],
                        e8[:, scp, i, :],
                        start=(nmm == 0),
                        stop=(nmm == SCc - 1),
                    )
                    nmm += 1
            # evacuate PSUM (DMA/gpsimd can't read PSUM), then an SBUF->SBUF
            # DMA shifts the 64 out rows into this head's stag partitions
            nc.vector.tensor_copy(oe_t[:, h, :], ps_o[0 : HD + 1, :])
            eng = nc.sync if h % 2 == 0 else nc.gpsimd
            eng.dma_start(
                stag_t[(h % 2) * HD : (h % 2 + 1) * HD, h // 2, :],
                oe_t[0:HD, h, :],
            )

        def den_chain(t, oe_t, stag_t, ot_t):
            # gather all 8 denominator rows into [128, 32] (flat (h, n)
            # order), invert once, bounce via DRAM to broadcast back
            dd = work.tile([P, NT // (2 * H)], mmdt, tag="dd")  # [128, 32]
            for c2 in range(4):
                eng = nc.gpsimd if c2 % 2 == 0 else nc.sync
                eng.dma_start(
                    dd[32 * c2 : 32 * (c2 + 1), :],
                    oe_t[HD : HD + 1, 2 * c2 : 2 * c2 + 2, :],
                )
            rd = work.tile([P, NT // (2 * H)], f32, tag="rd")
            rd16 = work.tile([P, NT // (2 * H)], mmdt, tag="rd16")
            nc.vector.reciprocal(rd[:], dd[:])
            nc.vector.tensor_copy(rd16[:], rd[:])
            nc.gpsimd.dma_start(rden_w[t], rd16[:])
            den_t = work.tile([P, IC, NT], mmdt, tag="den")
            for par in range(2):
                eng = nc.gpsimd if par == 0 else nc.sync
                eng.dma_start(
                    den_t[par * HD : (par + 1) * HD, :, :],
                    rden_r[t, par : par + 1, :, :].to_broadcast((HD, IC, NT)),
                )
            for c in range(IC):
                eng = nc.gpsimd if c % 2 == 0 else nc.vector
                eng.tensor_mul(ot_t[:, c, :], stag_t[:, c, :], den_t[:, c, :])

        def outproj_chunk(ot_t, t, ns):
            ps_y = psum.tile([P, D], f32, tag="ps_y", bufs=1)
            for c in range(IC):
                nc.tensor.matmul(
                    ps_y[:],
                    ot_t[:, c, ts(ns, P)],
                    wp_t[:, c, :],
                    start=(c == 0),
                    stop=(c == IC - 1),
                )
            y_t = work.tile([P, D], mmdt, tag="y")
            nc.vector.tensor_copy(y_t[:], ps_y[:])
            nc.sync.dma_start(
                y[t * NT + ns * P : t * NT + (ns + 1) * P, :], y_t[:]
            )

        # ---- main loop over query tiles ---------------------------------
        prev_ot = None  # ot of tile t-1 (outproj emitted as slot fillers)
        for t in range(NTILES):
            if t + 2 < NTILES:
                xT_fut = load_xT(t + 2)
            if t + 1 < NTILES:
                qm_next = work.tile([P, IC, NT], mmdt, tag="qm")
            stag_t = work.tile([P, IC, NT], mmdt, tag="stag")
            ot_t = work.tile([P, IC, NT], mmdt, tag="ot")
            oe_t = work.tile([HD + 1, H, NT], mmdt, tag="oe")
            e8s = [None] * H

            for h in range(H):
                e8s[h] = epool.tile([P, SCP, 2, NT], mmdt, tag="e", name="e8")
                scores_head(qm_cur, e8s[h], h)
                if h >= 1:
                    attnv_head(e8s[h - 1], oe_t, stag_t, h - 1)
                if h < IC:  # slots 0-3: qproj(t+1) filler
                    if t + 1 < NTILES:
                        qproj_chunk(xT_next, qm_next, h)
                else:  # slots 4-7: outproj(t-1) filler
                    if prev_ot is not None:
                        outproj_chunk(prev_ot, t - 1, h - IC)
            attnv_head(e8s[H - 1], oe_t, stag_t, H - 1)
            den_chain(t, oe_t, stag_t, ot_t)

            prev_ot = ot_t
            if t + 1 < NTILES:
                xT_cur, qm_cur = xT_next, qm_next
            if t + 2 < NTILES:
                xT_next = xT_fut

        for ns in range(NSUB):  # tail: outproj of the last tile
            outproj_chunk(prev_ot, NTILES - 1, ns)

    _split_multi_waits(nc)
    return nc


_NC_CACHE: dict = {}


def _get_nc(flags):
    if flags not in _NC_CACHE:
        _NC_CACHE[flags] = _build_nc(*flags)
    return _NC_CACHE[flags]


def _prep_in_maps(x, context, context_mask, wq, bq, wkv, bkv, wp, bp,
                  mmdt_name=None):
    if mmdt_name is None:
        mmdt_name = MMDT_NAME
    np_mm = _np_mm(getattr(mybir.dt, mmdt_name))
    cvt = lambda a: np.ascontiguousarray(a).astype(np_mm, copy=False)

    assert not (np.any(bq != 0) or np.any(bkv != 0) or np.any(bp != 0)), (
        "v2 kernel assumes all linear biases are zero"
    )

    # context compaction: unmasked positions first, truncate to the padded
    # max effective length over the batch (mask True = padding). Masked and
    # padded rows are ZEROED so they add nothing to scores/attnV, and the
    # indicator column keeps them out of the softmax denominator.
    n_eff = (~context_mask).sum(axis=1)
    Sc = int(min(S, max(2 * P, -(-int(n_eff.max()) // P) * P)))

    wqT = cvt(wq.T)
    wkT = cvt(wkv[:D].T)
    wvT = cvt(wkv[D:].T)
    wpT = cvt(wp.T)
    flags = (mmdt_name, Sc // P, PAIR_EXP)
    in_maps = []
    for b in range(B):
        sel = np.argsort(context_mask[b], kind="stable")[:Sc]
        ctx_c = np.ascontiguousarray(context[b][sel])
        ctx_c[int(n_eff[b]):] = 0.0
        uns_b = (~context_mask[b][sel]).astype(np.float32).reshape(Sc, 1)
        in_maps.append(
            {
                "xT": cvt(x[b].T),
                "ctxT": cvt(ctx_c.T),
                "wqT": wqT,
                "wkT": wkT,
                "wvT": wvT,
                "wpT": wpT,
                "uns": cvt(uns_b),
            }
        )
    return in_maps, flags


def kernel(x, context, context_mask, wq, bq, wkv, bkv, wp, bp):
    from concourse.bass_utils import run_bass_kernel_spmd

    in_maps, flags = _prep_in_maps(
        x, context, context_mask, wq, bq, wkv, bkv, wp, bp
    )
    nc = _get_nc(flags)
    res = run_bass_kernel_spmd(nc, in_maps, list(range(B)))
    return np.stack(
        [np.asarray(res.results[b]["y"]) for b in range(B)], axis=0
    ).astype(np.float32)
